# revision 6
# baseline (speedup 1.0000x reference)
"""Trainium2 Bass kernel for stacked ConvLSTM2D (4 layers, Keras semantics).

Scheme: space-to-depth s=2 block layout; each conv is a sum of block-tap
matmuls with K padded to 128 via shift-baked replica buffers (all matmuls
K=128, M=128, N=CR*Wb, bf16 in / f32 PSUM).

8-core SPMD: batch b on core pair (2b, 2b+1), split over image rows.
Odd cores solve a vertically-FLIPPED half (flipped x + flipped weight slabs
prepped on host) so the program is rank-symmetric: every core owns local
block rows 0..49 (L1: 0..59), its local top is a true image boundary, and
its exchange edge is its local bottom.
 - L1 runs 60 rows with no exchange: validity of the extra 10 overlap rows
   decays 1 row/step; after 10 steps rows <=50 are still valid, exactly
   what L2 needs.
 - L2..L4 exchange a 4-block-row halo every step: each core sends its rows
   46..49 (siy-swapped via two partition-block DMAs; channel order is
   siy-major so the swap is two contiguous partition ranges), pair
   AllReduce(add) in DRAM, then halo = sum - own (exact peer recovery),
   written row-reversed into frame rows 54..57.
 - hrep rebuild is split into main/halo DMAs so only the boundary chunk
   (ci=9) depends on the exchange; chunks 0..8 of the next timestep keep
   the PE busy while the collective flies.
Gates: M-order (i, f, g, o), gate blocks of 4*Fpad partitions, in-block
order (soy, f, sox). State c stays f32 in SBUF; h is bf16.
"""
import math
import os
from contextlib import ExitStack

import numpy as np
import ml_dtypes

import concourse.bacc as bacc
import concourse.bass as bass
import concourse.mybir as mybir
from concourse.tile import TileContext
from concourse.bass_utils import run_bass_kernel_spmd

BF16 = mybir.dt.bfloat16
F32 = mybir.dt.float32
AF = mybir.ActivationFunctionType
ALU = mybir.AluOpType

S = 2
PB = 4
# (cin_raw, F, k, tap_radius R)
LAYERS = [(1, 8, 3, 1), (8, 16, 5, 1), (16, 16, 9, 2), (16, 5, 12, 3)]
FPAD = [8, 16, 16, 8]
CINPAD = [1, 8, 16, 16]
N_CORES = 8
RG = [[0, 1], [2, 3], [4, 5], [6, 7]]

Wb = 100
WbP = Wb + 2 * PB          # 108
HBO1 = 60                  # L1 owned block rows per core
HP1 = HBO1 + 2 * PB        # 68
FLAT1 = HP1 * WbP          # 7344
HBO = 50                   # L2..L4 owned block rows per core
HP2 = HBO + 2 * PB         # 58
FLAT2 = HP2 * WbP          # 6264
HS = (PB + HBO) * WbP      # 5832, halo start (frame row 54)
CR = 5
T = 10


# ---------------------------------------------------------------- host prep --

def same_pad_lo(k):
    return (k - 1) // 2


def s2d_np(img):
    """[H, W, C] -> [4C, Hb, Wb], channel = siy*2C + c*2 + six (siy-major)."""
    H, W, C = img.shape
    Hb, Wbl = H // S, W // S
    t = img.reshape(Hb, S, Wbl, S, C)          # hb, siy, wb, six, c
    return t.transpose(1, 4, 3, 0, 2).reshape(4 * C, Hb, Wbl)


def un_s2d_np(blk, C, Cp, H, W):
    """blk [4Cp(siy,f,six), Hb, Wb] -> [H, W, C] (first C of Cp channels)."""
    Hb, Wbl = H // S, W // S
    b = blk.reshape(S, Cp, S, Hb, Wbl)[:, :C]  # siy, c, six, hb, wb
    return b.transpose(3, 0, 4, 2, 1).reshape(H, W, C)


def remap_kernel(Wk, cin_pad, F, Fp):
    k = Wk.shape[0]
    cin = Wk.shape[2]
    out = np.zeros((k, k, cin_pad, 4 * Fp), np.float32)
    for g in range(4):
        out[:, :, :cin, g * Fp:g * Fp + F] = Wk[:, :, :, g * F:(g + 1) * F]
    return out


def block_weights(Wk, pt, pl, R):
    """Wk [k,k,cinp,4Fp] -> dict[(by,bx)] of [4cinp, 16Fp] with
    row = siy*2cinp + ci*2 + six, col = g*4Fp + soy*2Fp + f*2 + sox."""
    k, _, cinp, coutp = Wk.shape
    Fp = coutp // 4
    out = {}
    for by in range(-R, R + 1):
        for bx in range(-R, R + 1):
            M = np.zeros((4 * cinp, 4 * coutp), np.float32)
            Mv = M.reshape(4 * cinp, 4, S, Fp, S)
            for siy in range(S):
                for six in range(S):
                    for soy in range(S):
                        for sox in range(S):
                            dy = S * by + siy - soy + pt
                            dx = S * bx + six - sox + pl
                            if 0 <= dy < k and 0 <= dx < k:
                                rows = slice(siy * 2 * cinp + six,
                                             siy * 2 * cinp + 2 * cinp, 2)
                                Mv[rows, :, soy, :, sox] = \
                                    Wk[dy, dx].reshape(cinp, 4, Fp)
            out[(by, bx)] = M
    return out


def conv_groups(li, conv):
    """Returns (K, reps, groups) where groups = [(bdy, bdx0)]."""
    R = LAYERS[li][3]
    K = 4 * (CINPAD[li] if conv == 'x' else FPAD[li])
    reps = 128 // K
    ngroups_x = math.ceil((2 * R + 1) / reps)
    groups = [(bdy, -R + m * reps)
              for bdy in range(-R, R + 1) for m in range(ngroups_x)]
    return K, reps, groups


def build_slabs(Wblk, li, conv):
    """-> np [nz*G, 128, 128] f32 (caller casts to bf16). Order: [zh][group]."""
    R = LAYERS[li][3]
    K, reps, groups = conv_groups(li, conv)
    nz = 2 if FPAD[li] == 16 else 1
    slabs = []
    for zh in range(nz):
        for (bdy, bdx0) in groups:
            slab = np.zeros((128, 128), np.float32)
            for j in range(reps):
                bdx = bdx0 + j
                if bdx > R:
                    continue
                slab[j * K:(j + 1) * K, :] = Wblk[(bdy, bdx)][:, zh * 128:(zh + 1) * 128]
            slabs.append(slab)
    return np.stack(slabs)


def prep_core_inputs(inputs, b, half):
    """Build the input map for core 2b+half (half 1 = vertically flipped)."""
    m = {}
    for li, (cin, F, k, R) in enumerate(LAYERS):
        pl = same_pad_lo(k)
        pt = pl if half == 0 else k - 1 - pl
        Fp = FPAD[li]
        Wx = np.asarray(inputs[f'Wx{li+1}'], np.float32)
        Wh = np.asarray(inputs[f'Wh{li+1}'], np.float32)
        if half == 1:
            Wx, Wh = Wx[::-1].copy(), Wh[::-1].copy()
        Wxb = block_weights(remap_kernel(Wx, CINPAD[li], F, Fp), pt, pl, R)
        Whb = block_weights(remap_kernel(Wh, Fp, F, Fp), pt, pl, R)
        braw = np.asarray(inputs[f'b{li+1}'], np.float32)
        bex = np.zeros(16 * Fp, np.float32)
        bexv = bex.reshape(4, S, Fp, S)
        for g in range(4):
            for f in range(F):
                bexv[g, :, f, :] = braw[g * F + f]
        NFp = 4 * Fp
        if Fp == 16:
            b1 = 0.2 * bex[:2 * NFp] + 0.5                             # (i,f) hsig'
            b2 = np.concatenate([bex[2 * NFp:3 * NFp],                 # g raw
                                 0.2 * bex[3 * NFp:] + 0.5])           # o hsig'
            m[f'bias{li+1}a'] = b1.reshape(128, 1)
            m[f'bias{li+1}b'] = b2.reshape(128, 1)
        else:
            b1 = np.concatenate([0.2 * bex[:2 * NFp] + 0.5,            # i,f
                                 bex[2 * NFp:3 * NFp],                 # g raw
                                 0.2 * bex[3 * NFp:] + 0.5])           # o
            m[f'bias{li+1}a'] = b1.reshape(128, 1)
        if li == 0:
            # L1 x: single K=36 im2col slab (tap-major rows), 1 group
            slab = np.zeros((128, 128), np.float32)
            for t_i, (bdy, bdx) in enumerate(
                    (by, bx) for by in range(-R, R + 1) for bx in range(-R, R + 1)):
                slab[t_i * 4:(t_i + 1) * 4, :] = Wxb[(bdy, bdx)]
            m['wx1'] = slab[None].astype(ml_dtypes.bfloat16)
        else:
            m[f'wx{li+1}'] = build_slabs(Wxb, li, 'x').astype(ml_dtypes.bfloat16)
        m[f'wh{li+1}'] = build_slabs(Whb, li, 'h').astype(ml_dtypes.bfloat16)

    # L1 x im2col: local 120 pixel rows (60 block rows), frame 68 block rows
    x = np.asarray(inputs['x'], np.float32)[b]          # [T, 200, 200, 1]
    if half == 1:
        x = x[:, ::-1]
    xc = np.zeros((T, 36, FLAT1 + 8), np.float32)
    for t in range(T):
        xp = np.zeros((4, HP1, WbP), np.float32)
        xp[:, PB:PB + HBO1, PB:PB + Wb] = s2d_np(x[t, 0:2 * HBO1])
        flat = xp.reshape(4, FLAT1)
        for t_i, (bdy, bdx) in enumerate(
                (by, bx) for by in (-1, 0, 1) for bx in (-1, 0, 1)):
            sh = bdy * WbP + bdx
            for c in range(4):
                if sh >= 0:
                    xc[t, t_i * 4 + c, :FLAT1 - sh] = flat[c, sh:]
                else:
                    xc[t, t_i * 4 + c, -sh:FLAT1] = flat[c, :FLAT1 + sh]
    m['xcol'] = xc.astype(ml_dtypes.bfloat16)
    return m


# ------------------------------------------------------------- kernel build --

def build_kernel(static_unroll=True):
    nc = bacc.Bacc("TRN2", target_bir_lowering=False, debug=False,
                   num_devices=N_CORES)

    xcol = nc.dram_tensor('xcol', [T, 36, FLAT1 + 8], BF16, kind="ExternalInput")
    wts, biases = {}, {}
    for li in range(4):
        nz = 2 if FPAD[li] == 16 else 1
        Kx, repx, gx = conv_groups(li, 'x')
        Kh, reph, gh = conv_groups(li, 'h')
        Gx = 1 if li == 0 else nz * len(gx)
        Gh = nz * len(gh)
        wts[(li, 'x')] = nc.dram_tensor(f'wx{li+1}', [Gx, 128, 128], BF16, kind="ExternalInput")
        wts[(li, 'h')] = nc.dram_tensor(f'wh{li+1}', [Gh, 128, 128], BF16, kind="ExternalInput")
        biases[(li, 'a')] = nc.dram_tensor(f'bias{li+1}a', [128, 1], F32, kind="ExternalInput")
        if nz == 2:
            biases[(li, 'b')] = nc.dram_tensor(f'bias{li+1}b', [128, 1], F32, kind="ExternalInput")
    hseqs = [nc.dram_tensor(f'hseq{li+1}', [T + 1, 4 * FPAD[li], FLAT2 + 8], BF16,
                            kind="Internal")
             for li in range(3)]
    out = nc.dram_tensor('out', [T, 32, HBO * Wb], F32, kind="ExternalOutput")

    with TileContext(nc) as tc, ExitStack() as top:
        gp = top.enter_context(tc.tile_pool(name="glob", bufs=1))
        xrep = gp.tile([128, FLAT1], BF16, tag="xrep")
        hrep = gp.tile([128, FLAT1], BF16, tag="hrep")
        nc.vector.memset(xrep[:, :], 0.0)

        for li in range(4):
            cin, F, k, R = LAYERS[li]
            Fp = FPAD[li]
            NFp = 4 * Fp
            nz = 2 if Fp == 16 else 1
            Kx, repx, gx = conv_groups(li, 'x')
            Kh, reph, gh = conv_groups(li, 'h')
            if li == 0:
                gx_list = [[(0, 0, 0)]]  # shifts baked into xcol data
            else:
                gx_list = [[(zh * len(gx) + i, bdy, bdx0)
                            for i, (bdy, bdx0) in enumerate(gx)] for zh in range(nz)]
            gh_list = [[(zh * len(gh) + i, bdy, bdx0)
                        for i, (bdy, bdx0) in enumerate(gh)] for zh in range(nz)]

            FLATl = FLAT1 if li == 0 else FLAT2
            HBOl = HBO1 if li == 0 else HBO
            NCHl = HBOl // CR

            with ExitStack() as ls:
                lp = ls.enter_context(tc.tile_pool(name=f"l{li}", bufs=1))
                pp = ls.enter_context(tc.tile_pool(name=f"ps{li}", bufs=4, space="PSUM"))
                tp = ls.enter_context(tc.tile_pool(name=f"tmp{li}", bufs=3))
                if li > 0:
                    dp = ls.enter_context(tc.tile_pool(name=f"xch{li}", bufs=2,
                                                       space="DRAM"))

                Gx = wts[(li, 'x')].shape[0]
                Gh = wts[(li, 'h')].shape[0]
                wxt = lp.tile([128, Gx * 128], BF16, tag="wx")
                wht = lp.tile([128, Gh * 128], BF16, tag="wh")
                nc.sync.dma_start(wxt[:, :].rearrange("p (g c) -> p g c", c=128),
                                  wts[(li, 'x')].ap().rearrange("g p c -> p g c"))
                nc.sync.dma_start(wht[:, :].rearrange("p (g c) -> p g c", c=128),
                                  wts[(li, 'h')].ap().rearrange("g p c -> p g c"))
                bia = lp.tile([128, 1], F32, tag="bia")
                nc.sync.dma_start(bia[:, :], biases[(li, 'a')].ap())
                if nz == 2:
                    bib = lp.tile([128, 1], F32, tag="bib")
                    nc.sync.dma_start(bib[:, :], biases[(li, 'b')].ap())

                H = lp.tile([NFp, FLATl + 8], BF16, tag="H")
                C = lp.tile([2 * NFp, HBOl, Wb], F32, tag="C")
                nc.vector.memset(H[:, :], 0.0)
                nc.vector.memset(C[:, :, :], 0.0)
                if li == 3:
                    OS = lp.tile([32, HBO, Wb], F32, tag="OS")
                if li < 3:
                    nc.sync.dma_start(hseqs[li].ap()[0, :, :], H[:, 0:FLAT2 + 8])
                if li > 0:
                    SB = lp.tile([NFp, 4, WbP], BF16, tag="SB")
                    Rt = lp.tile([NFp, 4, WbP], BF16, tag="Rt")

                H3 = H[:, 0:FLATl].rearrange("p (h w) -> p h w", w=WbP)

                def step_body(t):
                    # --- build XREP ---
                    if li == 0:
                        nc.sync.dma_start(xrep[0:36, 0:FLAT1],
                                          xcol.ap()[bass.ds(t, 1), :, 0:FLAT1])
                    else:
                        src = hseqs[li - 1].ap()
                        for j in range(repx):
                            nc.sync.dma_start(
                                xrep[j * Kx:(j + 1) * Kx, 0:FLAT2],
                                src[bass.ds(t + 1, 1), 0:Kx, j:j + FLAT2])
                    # --- build HREP from H (main / halo split for li>0) ---
                    if li == 0:
                        for j in range(reph):
                            nc.sync.dma_start(hrep[j * Kh:(j + 1) * Kh, 0:FLAT1],
                                              H[0:Kh, j:j + FLAT1])
                    else:
                        for j in range(reph):
                            nc.sync.dma_start(hrep[j * Kh:(j + 1) * Kh, 0:HS - 8],
                                              H[0:Kh, j:j + HS - 8])
                            nc.sync.dma_start(hrep[j * Kh:(j + 1) * Kh, HS - 8:FLAT2],
                                              H[0:Kh, HS - 8 + j:FLAT2 + j])
                    xr3 = xrep[:, 0:FLATl].rearrange("p (h w) -> p h w", w=WbP)
                    hr3 = hrep[:, 0:FLATl].rearrange("p (h w) -> p h w", w=WbP)

                    for ci in range(NCHl):
                        r0 = PB + ci * CR
                        zts = []
                        for zh in range(nz):
                            zt = pp.tile([128, CR, Wb], F32, tag="z")
                            mms = [(wxt, xr3, s, bdy, bdx0)
                                   for (s, bdy, bdx0) in gx_list[zh]] + \
                                  [(wht, hr3, s, bdy, bdx0)
                                   for (s, bdy, bdx0) in gh_list[zh]]
                            for mi, (wt, rep3, s, bdy, bdx0) in enumerate(mms):
                                nc.tensor.matmul(
                                    zt[:, :, :],
                                    wt[:, s * 128:(s + 1) * 128],
                                    rep3[:, r0 + bdy:r0 + bdy + CR,
                                         PB + bdx0:PB + bdx0 + Wb],
                                    start=(mi == 0), stop=(mi == len(mms) - 1))
                            zts.append(zt)

                        A = tp.tile([128, CR, Wb], F32, tag="A")
                        G = tp.tile([128, CR, Wb], F32, tag="G")
                        O = tp.tile([128, CR, Wb], F32, tag="O")
                        t1 = tp.tile([64, CR, Wb], F32, tag="t1")
                        t2 = tp.tile([64, CR, Wb], F32, tag="t2")
                        TC = tp.tile([128, CR, Wb], F32, tag="TC")
                        cw = C[:, ci * CR:(ci + 1) * CR, :]
                        hw = H3[0:NFp, r0:r0 + CR, PB:PB + Wb]
                        if nz == 2:
                            z1, z2 = zts
                            # A = hsig(z1*1 + b) over (i,f) [128]
                            nc.vector.tensor_scalar(A[:, :, :], z1[:, :, :], 0.2,
                                                    bia[:, 0:1], ALU.mult, ALU.add)
                            nc.vector.tensor_scalar(A[:, :, :], A[:, :, :], 0.0, 1.0,
                                                    ALU.max, ALU.min)
                            nc.scalar.activation(G[0:64, :, :], z2[0:64, :, :], AF.Tanh,
                                                 bias=bib[0:64, 0:1], scale=1.0)
                            nc.vector.tensor_scalar(O[64:128, :, :], z2[64:128, :, :], 0.2,
                                                    bib[64:128, 0:1], ALU.mult, ALU.add)
                            nc.vector.tensor_scalar(O[64:128, :, :], O[64:128, :, :],
                                                    0.0, 1.0, ALU.max, ALU.min)
                            nc.vector.tensor_tensor(t1[0:64, :, :], A[0:64, :, :],
                                                    G[0:64, :, :], ALU.mult)
                            nc.vector.tensor_tensor(t2[0:64, :, :], A[64:128, :, :],
                                                    cw[64:128, :, :], ALU.mult)
                            nc.vector.tensor_tensor(cw[64:128, :, :], t1[0:64, :, :],
                                                    t2[0:64, :, :], ALU.add)
                            nc.scalar.activation(TC[64:128, :, :], cw[64:128, :, :],
                                                 AF.Tanh)
                            if li == 3:
                                ow = OS[:, ci * CR:(ci + 1) * CR, :]
                                nc.vector.tensor_tensor(ow[:, :, :], O[64:128, :, :],
                                                        TC[64:128, :, :], ALU.mult)
                                nc.vector.tensor_copy(hw, ow[:, :, :])
                            else:
                                nc.vector.tensor_tensor(hw, O[64:128, :, :],
                                                        TC[64:128, :, :], ALU.mult)
                        else:
                            z = zts[0]
                            # layout (i,f,g,o) blocks of 32
                            nc.vector.tensor_scalar(A[0:64, :, :], z[0:64, :, :], 0.2,
                                                    bia[0:64, 0:1], ALU.mult, ALU.add)
                            nc.vector.tensor_scalar(A[0:64, :, :], A[0:64, :, :],
                                                    0.0, 1.0, ALU.max, ALU.min)
                            nc.scalar.activation(G[0:32, :, :], z[64:96, :, :], AF.Tanh,
                                                 bias=bia[64:96, 0:1], scale=1.0)
                            nc.vector.tensor_scalar(O[96:128, :, :], z[96:128, :, :], 0.2,
                                                    bia[96:128, 0:1], ALU.mult, ALU.add)
                            nc.vector.tensor_scalar(O[96:128, :, :], O[96:128, :, :],
                                                    0.0, 1.0, ALU.max, ALU.min)
                            nc.vector.tensor_tensor(t1[0:32, :, :], A[0:32, :, :],
                                                    G[0:32, :, :], ALU.mult)
                            nc.vector.tensor_tensor(t2[0:32, :, :], A[32:64, :, :],
                                                    cw[32:64, :, :], ALU.mult)
                            nc.vector.tensor_tensor(cw[32:64, :, :], t1[0:32, :, :],
                                                    t2[0:32, :, :], ALU.add)
                            nc.scalar.activation(TC[96:128, :, :], cw[32:64, :, :],
                                                 AF.Tanh)
                            if li == 3:
                                ow = OS[:, ci * CR:(ci + 1) * CR, :]
                                nc.vector.tensor_tensor(ow[:, :, :], O[96:128, :, :],
                                                        TC[96:128, :, :], ALU.mult)
                                nc.vector.tensor_copy(hw, ow[:, :, :])
                            else:
                                nc.vector.tensor_tensor(hw, O[96:128, :, :],
                                                        TC[96:128, :, :], ALU.mult)

                    # --- halo exchange (L2..L4, skip last step of L4) ---
                    if li > 0 and not (li == 3 and t == T - 1):
                        TFp = 2 * Fp
                        nc.sync.dma_start(SB[0:TFp, :, :],
                                          H3[TFp:NFp, PB + HBO - 4:PB + HBO, :])
                        nc.sync.dma_start(SB[TFp:NFp, :, :],
                                          H3[0:TFp, PB + HBO - 4:PB + HBO, :])
                        sendt = dp.tile([NFp, 4 * WbP], BF16, tag="send")
                        recvt = dp.tile([NFp, 4 * WbP], BF16, tag="recv")
                        nc.sync.dma_start(sendt[:, :],
                                          SB[:, :, :].rearrange("p h w -> p (h w)"))
                        nc.gpsimd.collective_compute(
                            "AllReduce", ALU.add, replica_groups=RG,
                            ins=[sendt[:, :]], outs=[recvt[:, :]])
                        nc.sync.dma_start(Rt[:, :, :].rearrange("p h w -> p (h w)"),
                                          recvt[:, :])
                        for r in range(4):
                            nc.vector.tensor_tensor(
                                H3[:, PB + HBO + r, :], Rt[:, 3 - r, :],
                                SB[:, 3 - r, :], ALU.subtract)

                    if li < 3:
                        nc.sync.dma_start(hseqs[li].ap()[bass.ds(t + 1, 1), :, :],
                                          H[:, 0:FLAT2 + 8])
                    else:
                        nc.sync.dma_start(
                            out.ap()[bass.ds(t, 1), :, :],
                            OS[:, :, :].rearrange("p h w -> p (h w)"))

                if static_unroll:
                    for t in range(T):
                        step_body(t)
                else:
                    with tc.For_i(0, T) as t:
                        step_body(t)
    nc.compile()
    return nc


# ------------------------------------------------------------------ runner --

_CACHED = {}
LAST_EXEC_NS = None


def kernel(**inputs) -> np.ndarray:
    x = np.asarray(inputs['x'])
    B, Tt, Hf, Wf, _ = x.shape
    assert (Tt, Hf, Wf) == (T, 200, 200)
    if 'nc' not in _CACHED:
        _CACHED['nc'] = build_kernel(static_unroll=True)
    nc = _CACHED['nc']
    in_maps = [prep_core_inputs(inputs, b, half)
               for b in range(B) for half in range(2)]
    res = run_bass_kernel_spmd(nc, in_maps, core_ids=list(range(N_CORES)))
    global LAST_EXEC_NS
    LAST_EXEC_NS = res.exec_time_ns
    outs = np.zeros((B, T, 2 * S * HBO, S * Wb, 5), np.float32)
    for b in range(B):
        for half in range(2):
            o = res.results[2 * b + half]['out']   # [T, 32, HBO*Wb]
            for t in range(T):
                img = un_s2d_np(o[t].reshape(32, HBO, Wb), 5, FPAD[3],
                                S * HBO, S * Wb)
                if half == 0:
                    outs[b, t, 0:100] = img
                else:
                    outs[b, t, 100:200] = img[::-1]
    if os.environ.get('KERNEL_TIME'):
        LAST_EXEC_NS = _timed_run(nc, in_maps,
                                  iters=int(os.environ.get('KERNEL_TIME_ITERS', '5')))
    return outs


def _timed_run(nc, in_maps, iters=5):
    """Wall-clock the NEFF execution via a non-donating jitted shard_map,
    device-resident inputs, min over iters. Returns ns."""
    import time
    import jax
    from jax.sharding import Mesh, PartitionSpec, NamedSharding
    from jax.experimental.shard_map import shard_map
    from concourse import bass2jax as b2j

    b2j.install_neuronx_cc_hook()
    partition_name = (nc.partition_id_tensor.name
                      if nc.partition_id_tensor else None)
    in_names, out_names, out_avals, zero_outs = [], [], [], []
    for alloc in nc.m.functions[0].allocations:
        if not isinstance(alloc, mybir.MemoryLocationSet):
            continue
        name = alloc.memorylocations[0].name
        if alloc.kind == "ExternalInput":
            if name != partition_name:
                in_names.append(name)
        elif alloc.kind == "ExternalOutput":
            shape = tuple(alloc.tensor_shape)
            npdt = mybir.dt.np(alloc.dtype)
            out_names.append(name)
            out_avals.append(jax.core.ShapedArray(shape, npdt))
            zero_outs.append(np.zeros(shape, npdt))
    n_params = len(in_names)
    in_names = in_names + out_names
    if partition_name is not None:
        in_names.append(partition_name)

    def _body(*args):
        operands = list(args)
        if partition_name is not None:
            operands.append(b2j.partition_id_tensor())
        outs = b2j._bass_exec_p.bind(
            *operands, out_avals=tuple(out_avals), in_names=tuple(in_names),
            out_names=tuple(out_names), lowering_input_output_aliases=(),
            sim_require_finite=True, sim_require_nnan=True, nc=nc)
        return tuple(outs)

    n = len(in_maps)
    devices = jax.devices()[:n]
    mesh = Mesh(np.asarray(devices), ("core",))
    f = jax.jit(shard_map(_body, mesh=mesh,
                          in_specs=(PartitionSpec("core"),) * (n_params + len(out_names)),
                          out_specs=(PartitionSpec("core"),) * len(out_names),
                          check_rep=False),
                keep_unused=True)
    sh = NamedSharding(mesh, PartitionSpec("core"))
    args = [jax.device_put(
                np.concatenate([np.asarray(in_maps[c][nm]) for c in range(n)], axis=0), sh)
            for nm in in_names[:n_params]]
    args += [jax.device_put(np.concatenate([z] * n, axis=0), sh) for z in zero_outs]
    ts = []
    for _ in range(iters + 1):
        t0 = time.perf_counter()
        o = f(*args)
        jax.block_until_ready(o)
        ts.append(time.perf_counter() - t0)
    best = min(ts[1:])
    print(f'timed_run wall times (s): {[f"{x:.4f}" for x in ts]}', flush=True)
    return int(best * 1e9)


# revision 14
# speedup vs baseline: 11.7857x; 11.7857x over previous
"""Trainium2 Bass kernel for stacked ConvLSTM2D (4 layers, Keras semantics).

Scheme: space-to-depth s=2 block layout; each conv is a sum of block-tap
matmuls with K padded to 128 via shift-baked replica buffers (all matmuls
K=128, M=128, N=CR*Wb, bf16 in / f32 PSUM).

8-core SPMD: batch b on core pair (2b, 2b+1), split over image rows.
Odd cores solve a vertically-FLIPPED half (flipped x + flipped weight slabs
prepped on host) so the program is rank-symmetric: every core owns local
block rows 0..49 (L1: 0..59), its local top is a true image boundary, and
its exchange edge is its local bottom.
 - L1 runs 60 rows with no exchange: validity of the extra 10 overlap rows
   decays 1 row/step; after 10 steps rows <=50 are still valid, exactly
   what L2 needs.
 - L2..L4 exchange a 4-block-row halo every step: each core sends its rows
   46..49 (siy-swapped via two partition-block DMAs; channel order is
   siy-major so the swap is two contiguous partition ranges), pair
   AllReduce(add) in DRAM, then halo = sum - own (exact peer recovery),
   written row-reversed into frame rows 54..57.
 - hrep rebuild is split into main/halo DMAs so only the boundary chunk
   (ci=9) depends on the exchange; chunks 0..8 of the next timestep keep
   the PE busy while the collective flies.
Gates: M-order (i, f, g, o), gate blocks of 4*Fpad partitions, in-block
order (soy, f, sox). State c stays f32 in SBUF; h is bf16.
"""
import math
import os
from contextlib import ExitStack

import numpy as np
import ml_dtypes

import concourse.bacc as bacc
import concourse.bass as bass
import concourse.mybir as mybir
from concourse.tile import TileContext
from concourse.bass_utils import run_bass_kernel_spmd

BF16 = mybir.dt.bfloat16
F32 = mybir.dt.float32
AF = mybir.ActivationFunctionType
ALU = mybir.AluOpType

S = 2
PB = 4
# (cin_raw, F, k, tap_radius R)
LAYERS = [(1, 8, 3, 1), (8, 16, 5, 1), (16, 16, 9, 2), (16, 5, 12, 3)]
FPAD = [8, 16, 16, 8]
CINPAD = [1, 8, 16, 16]
N_CORES = 8
RG = [[0, 1], [2, 3], [4, 5], [6, 7]]

Wb = 100
WbP = Wb + 2 * PB          # 108
HBO1 = 60                  # L1 owned block rows per core
HP1 = HBO1 + 2 * PB        # 68
FLAT1 = HP1 * WbP          # 7344
HBO = 50                   # L2..L4 owned block rows per core
HP2 = HBO + 2 * PB         # 58
FLAT2 = HP2 * WbP          # 6264
HS = (PB + HBO) * WbP      # 5832, halo start (frame row 54)
CR = 5
T = 10


# ---------------------------------------------------------------- host prep --

def same_pad_lo(k):
    return (k - 1) // 2


def s2d_np(img):
    """[H, W, C] -> [4C, Hb, Wb], channel = siy*2C + c*2 + six (siy-major)."""
    H, W, C = img.shape
    Hb, Wbl = H // S, W // S
    t = img.reshape(Hb, S, Wbl, S, C)          # hb, siy, wb, six, c
    return t.transpose(1, 4, 3, 0, 2).reshape(4 * C, Hb, Wbl)


def un_s2d_np(blk, C, Cp, H, W):
    """blk [4Cp(siy,f,six), Hb, Wb] -> [H, W, C] (first C of Cp channels)."""
    Hb, Wbl = H // S, W // S
    b = blk.reshape(S, Cp, S, Hb, Wbl)[:, :C]  # siy, c, six, hb, wb
    return b.transpose(3, 0, 4, 2, 1).reshape(H, W, C)


def remap_kernel(Wk, cin_pad, F, Fp):
    k = Wk.shape[0]
    cin = Wk.shape[2]
    out = np.zeros((k, k, cin_pad, 4 * Fp), np.float32)
    for g in range(4):
        out[:, :, :cin, g * Fp:g * Fp + F] = Wk[:, :, :, g * F:(g + 1) * F]
    return out


def block_weights(Wk, pt, pl, R):
    """Wk [k,k,cinp,4Fp] -> dict[(by,bx)] of [4cinp, 16Fp] with
    row = siy*2cinp + ci*2 + six, col = g*4Fp + soy*2Fp + f*2 + sox."""
    k, _, cinp, coutp = Wk.shape
    Fp = coutp // 4
    out = {}
    for by in range(-R, R + 1):
        for bx in range(-R, R + 1):
            M = np.zeros((4 * cinp, 4 * coutp), np.float32)
            Mv = M.reshape(4 * cinp, 4, S, Fp, S)
            for siy in range(S):
                for six in range(S):
                    for soy in range(S):
                        for sox in range(S):
                            dy = S * by + siy - soy + pt
                            dx = S * bx + six - sox + pl
                            if 0 <= dy < k and 0 <= dx < k:
                                rows = slice(siy * 2 * cinp + six,
                                             siy * 2 * cinp + 2 * cinp, 2)
                                Mv[rows, :, soy, :, sox] = \
                                    Wk[dy, dx].reshape(cinp, 4, Fp)
            out[(by, bx)] = M
    return out


def conv_groups(li, conv):
    """Returns (K, reps, groups) where groups = [(bdy, bdx0)]."""
    R = LAYERS[li][3]
    K = 4 * (CINPAD[li] if conv == 'x' else FPAD[li])
    reps = 128 // K
    ngroups_x = math.ceil((2 * R + 1) / reps)
    groups = [(bdy, -R + m * reps)
              for bdy in range(-R, R + 1) for m in range(ngroups_x)]
    return K, reps, groups


def build_slabs(Wblk, li, conv):
    """-> np [nz*G, 128, 128] f32 (caller casts to bf16). Order: [zh][group]."""
    R = LAYERS[li][3]
    K, reps, groups = conv_groups(li, conv)
    nz = 2 if FPAD[li] == 16 else 1
    slabs = []
    for zh in range(nz):
        for (bdy, bdx0) in groups:
            slab = np.zeros((128, 128), np.float32)
            for j in range(reps):
                bdx = bdx0 + j
                if bdx > R:
                    continue
                slab[j * K:(j + 1) * K, :] = Wblk[(bdy, bdx)][:, zh * 128:(zh + 1) * 128]
            slabs.append(slab)
    return np.stack(slabs)


def prep_core_inputs(inputs, b, half):
    """Build the input map for core 2b+half (half 1 = vertically flipped)."""
    m = {}
    for li, (cin, F, k, R) in enumerate(LAYERS):
        pl = same_pad_lo(k)
        pt = pl if half == 0 else k - 1 - pl
        Fp = FPAD[li]
        Wx = np.asarray(inputs[f'Wx{li+1}'], np.float32)
        Wh = np.asarray(inputs[f'Wh{li+1}'], np.float32)
        if half == 1:
            Wx, Wh = Wx[::-1].copy(), Wh[::-1].copy()
        Wxb = block_weights(remap_kernel(Wx, CINPAD[li], F, Fp), pt, pl, R)
        Whb = block_weights(remap_kernel(Wh, Fp, F, Fp), pt, pl, R)
        braw = np.asarray(inputs[f'b{li+1}'], np.float32)
        bex = np.zeros(16 * Fp, np.float32)
        bexv = bex.reshape(4, S, Fp, S)
        for g in range(4):
            for f in range(F):
                bexv[g, :, f, :] = braw[g * F + f]
        NFp = 4 * Fp
        if Fp == 16:
            b1 = 0.2 * bex[:2 * NFp] + 0.5                             # (i,f) hsig'
            b2 = np.concatenate([bex[2 * NFp:3 * NFp],                 # g raw
                                 0.2 * bex[3 * NFp:] + 0.5])           # o hsig'
            m[f'bias{li+1}a'] = b1.reshape(128, 1)
            m[f'bias{li+1}b'] = b2.reshape(128, 1)
        else:
            b1 = np.concatenate([0.2 * bex[:2 * NFp] + 0.5,            # i,f
                                 bex[2 * NFp:3 * NFp],                 # g raw
                                 0.2 * bex[3 * NFp:] + 0.5])           # o
            m[f'bias{li+1}a'] = b1.reshape(128, 1)
        if li == 0:
            # L1 x: single K=36 im2col slab (tap-major rows), 1 group
            slab = np.zeros((128, 128), np.float32)
            for t_i, (bdy, bdx) in enumerate(
                    (by, bx) for by in range(-R, R + 1) for bx in range(-R, R + 1)):
                slab[t_i * 4:(t_i + 1) * 4, :] = Wxb[(bdy, bdx)]
            m['wx1'] = slab[None].astype(ml_dtypes.bfloat16)
        else:
            m[f'wx{li+1}'] = build_slabs(Wxb, li, 'x').astype(ml_dtypes.bfloat16)
        m[f'wh{li+1}'] = build_slabs(Whb, li, 'h').astype(ml_dtypes.bfloat16)

    # L1 x im2col: local 120 pixel rows (60 block rows), frame 68 block rows
    x = np.asarray(inputs['x'], np.float32)[b]          # [T, 200, 200, 1]
    if half == 1:
        x = x[:, ::-1]
    xc = np.zeros((T, 36, FLAT1 + 8), np.float32)
    for t in range(T):
        xp = np.zeros((4, HP1, WbP), np.float32)
        xp[:, PB:PB + HBO1, PB:PB + Wb] = s2d_np(x[t, 0:2 * HBO1])
        flat = xp.reshape(4, FLAT1)
        for t_i, (bdy, bdx) in enumerate(
                (by, bx) for by in (-1, 0, 1) for bx in (-1, 0, 1)):
            sh = bdy * WbP + bdx
            for c in range(4):
                if sh >= 0:
                    xc[t, t_i * 4 + c, :FLAT1 - sh] = flat[c, sh:]
                else:
                    xc[t, t_i * 4 + c, -sh:FLAT1] = flat[c, :FLAT1 + sh]
    m['xcol'] = xc.astype(ml_dtypes.bfloat16)
    return m


# ------------------------------------------------------------- kernel build --

def build_kernel(static_unroll=True):
    nc = bacc.Bacc("TRN2", target_bir_lowering=False, debug=False,
                   num_devices=N_CORES)

    xcol = nc.dram_tensor('xcol', [T, 36, FLAT1 + 8], BF16, kind="ExternalInput")
    wts, biases = {}, {}
    for li in range(4):
        nz = 2 if FPAD[li] == 16 else 1
        Kx, repx, gx = conv_groups(li, 'x')
        Kh, reph, gh = conv_groups(li, 'h')
        Gx = 1 if li == 0 else nz * len(gx)
        Gh = nz * len(gh)
        wts[(li, 'x')] = nc.dram_tensor(f'wx{li+1}', [Gx, 128, 128], BF16, kind="ExternalInput")
        wts[(li, 'h')] = nc.dram_tensor(f'wh{li+1}', [Gh, 128, 128], BF16, kind="ExternalInput")
        biases[(li, 'a')] = nc.dram_tensor(f'bias{li+1}a', [128, 1], F32, kind="ExternalInput")
        if nz == 2:
            biases[(li, 'b')] = nc.dram_tensor(f'bias{li+1}b', [128, 1], F32, kind="ExternalInput")
    hseqs = [nc.dram_tensor(f'hseq{li+1}', [T + 1, 4 * FPAD[li], FLAT2 + 8], BF16,
                            kind="Internal")
             for li in range(3)]
    out = nc.dram_tensor('out', [T, 32, HBO * Wb], F32, kind="ExternalOutput")

    with TileContext(nc) as tc, ExitStack() as top:
        gp = top.enter_context(tc.tile_pool(name="glob", bufs=1))
        xrA = gp.tile([128, FLAT1], BF16, tag="xrA")
        xrB = gp.tile([128, FLAT1], BF16, tag="xrB")
        hrA = gp.tile([128, FLAT1], BF16, tag="hrA")
        hrB = gp.tile([128, FLAT1], BF16, tag="hrB")
        nc.vector.memset(xrA[:, :], 0.0)
        nc.vector.memset(xrB[:, :], 0.0)

        for li in range(4):
            cin, F, k, R = LAYERS[li]
            Fp = FPAD[li]
            NFp = 4 * Fp
            nz = 2 if Fp == 16 else 1
            Kx, repx, gx = conv_groups(li, 'x')
            Kh, reph, gh = conv_groups(li, 'h')
            if li == 0:
                gx_list = [[(0, 0, 0)]]  # shifts baked into xcol data
            else:
                gx_list = [[(zh * len(gx) + i, bdy, bdx0)
                            for i, (bdy, bdx0) in enumerate(gx)] for zh in range(nz)]
            gh_list = [[(zh * len(gh) + i, bdy, bdx0)
                        for i, (bdy, bdx0) in enumerate(gh)] for zh in range(nz)]

            FLATl = FLAT1 if li == 0 else FLAT2
            HBOl = HBO1 if li == 0 else HBO
            NCHl = HBOl // CR

            with ExitStack() as ls:
                lp = ls.enter_context(tc.tile_pool(name=f"l{li}", bufs=1))
                pp = ls.enter_context(tc.tile_pool(name=f"ps{li}", bufs=4, space="PSUM"))
                tp = ls.enter_context(tc.tile_pool(name=f"tmp{li}", bufs=3))
                if li > 0:
                    dp = ls.enter_context(tc.tile_pool(name=f"xch{li}", bufs=2,
                                                       space="DRAM"))

                Gx = wts[(li, 'x')].shape[0]
                Gh = wts[(li, 'h')].shape[0]
                wxt = lp.tile([128, Gx * 128], BF16, tag="wx")
                wht = lp.tile([128, Gh * 128], BF16, tag="wh")
                nc.sync.dma_start(wxt[:, :].rearrange("p (g c) -> p g c", c=128),
                                  wts[(li, 'x')].ap().rearrange("g p c -> p g c"))
                nc.sync.dma_start(wht[:, :].rearrange("p (g c) -> p g c", c=128),
                                  wts[(li, 'h')].ap().rearrange("g p c -> p g c"))
                bia = lp.tile([128, 1], F32, tag="bia")
                nc.sync.dma_start(bia[:, :], biases[(li, 'a')].ap())
                if nz == 2:
                    bib = lp.tile([128, 1], F32, tag="bib")
                    nc.sync.dma_start(bib[:, :], biases[(li, 'b')].ap())

                H = lp.tile([NFp, FLATl + 8], BF16, tag="H")
                C = lp.tile([2 * NFp, HBOl, Wb], F32, tag="C")
                nc.vector.memset(H[:, :], 0.0)
                nc.vector.memset(C[:, :, :], 0.0)
                if li == 3:
                    OS = lp.tile([32, HBO, Wb], F32, tag="OS")
                if li < 3:
                    nc.sync.dma_start(hseqs[li].ap()[0, :, :], H[:, 0:FLAT2 + 8])
                if li > 0:
                    SB = lp.tile([NFp, 4, WbP], BF16, tag="SB")
                    Rt = lp.tile([NFp, 4, WbP], BF16, tag="Rt")

                H3 = H[:, 0:FLATl].rearrange("p (h w) -> p h w", w=WbP)

                def build_xr(t, buf):
                    if li == 0:
                        nc.sync.dma_start(buf[0:36, 0:FLAT1],
                                          xcol.ap()[bass.ds(t, 1), :, 0:FLAT1])
                    else:
                        src = hseqs[li - 1].ap()
                        for j in range(repx):
                            nc.sync.dma_start(
                                buf[j * Kx:(j + 1) * Kx, 0:FLAT2],
                                src[bass.ds(t + 1, 1), 0:Kx, j:j + FLAT2])

                def band_range(b):
                    lo = 0 if b == 0 else (PB + b * CR) * WbP
                    if b == NCHl - 1:
                        hi = FLAT1 if li == 0 else HS - 8
                    else:
                        hi = (PB + (b + 1) * CR) * WbP
                    return lo, hi

                def issue_band(b, buf):
                    lo, hi = band_range(b)
                    for j in range(reph):
                        nc.sync.dma_start(buf[j * Kh:(j + 1) * Kh, lo:hi],
                                          H[0:Kh, lo + j:hi + j])

                # initial state (t=0): hrA holds zeros; xrA holds x/hseq slice 0
                nc.vector.memset(hrA[:, :], 0.0)
                build_xr(0, xrA)

                def step_body(t):
                    xr_cur, xr_nxt = (xrA, xrB) if t % 2 == 0 else (xrB, xrA)
                    hr_cur, hr_nxt = (hrA, hrB) if t % 2 == 0 else (hrB, hrA)
                    if t < T - 1:
                        build_xr(t + 1, xr_nxt)
                    xr3 = xr_cur[:, 0:FLATl].rearrange("p (h w) -> p h w", w=WbP)
                    hr3 = hr_cur[:, 0:FLATl].rearrange("p (h w) -> p h w", w=WbP)

                    for ci in range(NCHl):
                        r0 = PB + ci * CR
                        zts = []
                        for zh in range(nz):
                            zt = pp.tile([128, CR, Wb], F32, tag="z")
                            mms = [(wxt, xr3, s, bdy, bdx0)
                                   for (s, bdy, bdx0) in gx_list[zh]] + \
                                  [(wht, hr3, s, bdy, bdx0)
                                   for (s, bdy, bdx0) in gh_list[zh]]
                            for mi, (wt, rep3, s, bdy, bdx0) in enumerate(mms):
                                nc.tensor.matmul(
                                    zt[:, :, :],
                                    wt[:, s * 128:(s + 1) * 128],
                                    rep3[:, r0 + bdy:r0 + bdy + CR,
                                         PB + bdx0:PB + bdx0 + Wb],
                                    start=(mi == 0), stop=(mi == len(mms) - 1))
                            zts.append(zt)

                        A = tp.tile([128, CR, Wb], F32, tag="A")
                        G = tp.tile([128, CR, Wb], F32, tag="G")
                        O = tp.tile([128, CR, Wb], F32, tag="O")
                        t1 = tp.tile([64, CR, Wb], F32, tag="t1")
                        t2 = tp.tile([64, CR, Wb], F32, tag="t2")
                        TC = tp.tile([128, CR, Wb], F32, tag="TC")
                        cw = C[:, ci * CR:(ci + 1) * CR, :]
                        hw = H3[0:NFp, r0:r0 + CR, PB:PB + Wb]
                        if nz == 2:
                            z1, z2 = zts
                            # A = hsig(z1*1 + b) over (i,f) [128]
                            nc.vector.tensor_scalar(A[:, :, :], z1[:, :, :], 0.2,
                                                    bia[:, 0:1], ALU.mult, ALU.add)
                            nc.vector.tensor_scalar(A[:, :, :], A[:, :, :], 0.0, 1.0,
                                                    ALU.max, ALU.min)
                            nc.scalar.activation(G[0:64, :, :], z2[0:64, :, :], AF.Tanh,
                                                 bias=bib[0:64, 0:1], scale=1.0)
                            nc.vector.tensor_scalar(O[64:128, :, :], z2[64:128, :, :], 0.2,
                                                    bib[64:128, 0:1], ALU.mult, ALU.add)
                            nc.vector.tensor_scalar(O[64:128, :, :], O[64:128, :, :],
                                                    0.0, 1.0, ALU.max, ALU.min)
                            nc.vector.tensor_tensor(t1[0:64, :, :], A[0:64, :, :],
                                                    G[0:64, :, :], ALU.mult)
                            nc.vector.tensor_tensor(t2[0:64, :, :], A[64:128, :, :],
                                                    cw[64:128, :, :], ALU.mult)
                            nc.vector.tensor_tensor(cw[64:128, :, :], t1[0:64, :, :],
                                                    t2[0:64, :, :], ALU.add)
                            nc.scalar.activation(TC[64:128, :, :], cw[64:128, :, :],
                                                 AF.Tanh)
                            if li == 3:
                                ow = OS[:, ci * CR:(ci + 1) * CR, :]
                                nc.vector.tensor_tensor(ow[:, :, :], O[64:128, :, :],
                                                        TC[64:128, :, :], ALU.mult)
                                nc.vector.tensor_copy(hw, ow[:, :, :])
                            else:
                                nc.vector.tensor_tensor(hw, O[64:128, :, :],
                                                        TC[64:128, :, :], ALU.mult)
                        else:
                            z = zts[0]
                            # layout (i,f,g,o) blocks of 32
                            nc.vector.tensor_scalar(A[0:64, :, :], z[0:64, :, :], 0.2,
                                                    bia[0:64, 0:1], ALU.mult, ALU.add)
                            nc.vector.tensor_scalar(A[0:64, :, :], A[0:64, :, :],
                                                    0.0, 1.0, ALU.max, ALU.min)
                            nc.scalar.activation(G[0:32, :, :], z[64:96, :, :], AF.Tanh,
                                                 bias=bia[64:96, 0:1], scale=1.0)
                            nc.vector.tensor_scalar(O[96:128, :, :], z[96:128, :, :], 0.2,
                                                    bia[96:128, 0:1], ALU.mult, ALU.add)
                            nc.vector.tensor_scalar(O[96:128, :, :], O[96:128, :, :],
                                                    0.0, 1.0, ALU.max, ALU.min)
                            nc.vector.tensor_tensor(t1[0:32, :, :], A[0:32, :, :],
                                                    G[0:32, :, :], ALU.mult)
                            nc.vector.tensor_tensor(t2[0:32, :, :], A[32:64, :, :],
                                                    cw[32:64, :, :], ALU.mult)
                            nc.vector.tensor_tensor(cw[32:64, :, :], t1[0:32, :, :],
                                                    t2[0:32, :, :], ALU.add)
                            nc.scalar.activation(TC[96:128, :, :], cw[32:64, :, :],
                                                 AF.Tanh)
                            if li == 3:
                                ow = OS[:, ci * CR:(ci + 1) * CR, :]
                                nc.vector.tensor_tensor(ow[:, :, :], O[96:128, :, :],
                                                        TC[96:128, :, :], ALU.mult)
                                nc.vector.tensor_copy(hw, ow[:, :, :])
                            else:
                                nc.vector.tensor_tensor(hw, O[96:128, :, :],
                                                        TC[96:128, :, :], ALU.mult)
                        if t < T - 1 and ci >= 1:
                            issue_band(ci - 1, hr_nxt)
                    if t < T - 1:
                        issue_band(NCHl - 1, hr_nxt)

                    if li == 3:
                        nc.sync.dma_start(
                            out.ap()[bass.ds(t, 1), :, :],
                            OS[:, :, :].rearrange("p h w -> p (h w)"))
                    elif li == 0:
                        nc.sync.dma_start(hseqs[li].ap()[bass.ds(t + 1, 1), :, :],
                                          H[:, 0:FLAT2 + 8])
                    else:
                        # main part of hseq (no halo dependency)
                        nc.sync.dma_start(
                            hseqs[li].ap()[bass.ds(t + 1, 1), :, 0:HS - 8],
                            H[:, 0:HS - 8])

                    # --- halo exchange (L2..L4, skip last step of L4) ---
                    if li > 0 and not (li == 3 and t == T - 1):
                        TFp = 2 * Fp
                        nc.gpsimd.dma_start(SB[0:TFp, :, :],
                                            H3[TFp:NFp, PB + HBO - 4:PB + HBO, :])
                        nc.gpsimd.dma_start(SB[TFp:NFp, :, :],
                                            H3[0:TFp, PB + HBO - 4:PB + HBO, :])
                        sendt = dp.tile([NFp, 4 * WbP], BF16, tag="send")
                        recvt = dp.tile([NFp, 4 * WbP], BF16, tag="recv")
                        nc.gpsimd.dma_start(sendt[:, :],
                                            SB[:, :, :].rearrange("p h w -> p (h w)"))
                        nc.gpsimd.collective_compute(
                            "AllReduce", ALU.add, replica_groups=RG,
                            ins=[sendt[:, :]], outs=[recvt[:, :]])
                        nc.gpsimd.dma_start(Rt[:, :, :].rearrange("p h w -> p (h w)"),
                                            recvt[:, :])
                        for r in range(4):
                            nc.vector.tensor_tensor(
                                H3[:, PB + HBO + r, :], Rt[:, 3 - r, :],
                                SB[:, 3 - r, :], ALU.subtract)
                        # halo band of hrep for t+1 + halo part of hseq store
                        if t < T - 1:
                            for j in range(reph):
                                nc.gpsimd.dma_start(
                                    hr_nxt[j * Kh:(j + 1) * Kh, HS - 8:FLAT2],
                                    H[0:Kh, HS - 8 + j:FLAT2 + j])
                        if li < 3:
                            nc.gpsimd.dma_start(
                                hseqs[li].ap()[bass.ds(t + 1, 1), :, HS - 8:FLAT2 + 8],
                                H[:, HS - 8:FLAT2 + 8])

                if static_unroll:
                    for t in range(T):
                        step_body(t)
                else:
                    with tc.For_i(0, T) as t:
                        step_body(t)
    nc.compile()
    return nc


# ------------------------------------------------------------------ runner --

_CACHED = {}
LAST_EXEC_NS = None


def _install_ntff_hook():
    """Provide the antenv.axon_hooks module this image lacks, backed by
    ctypes calls into libaxon_pjrt.so (same ABI trn_boot would use)."""
    import sys
    import types
    import ctypes
    import contextlib
    if 'antenv.axon_hooks' in sys.modules:
        return True
    try:
        lib = ctypes.CDLL('/opt/axon/libaxon_pjrt.so')
    except OSError:
        return False
    if not hasattr(lib, 'axon_start_nrt_profile'):
        return False
    lib.axon_start_nrt_profile.argtypes = [ctypes.POINTER(ctypes.c_int64),
                                           ctypes.c_size_t]
    lib.axon_start_nrt_profile.restype = ctypes.c_int64
    lib.axon_stop_nrt_profile.argtypes = [ctypes.c_char_p]
    lib.axon_stop_nrt_profile.restype = ctypes.c_int64

    @contextlib.contextmanager
    def _hook(output_dir, device_ids):
        import jax
        jax.devices()
        if device_ids:
            ids = (ctypes.c_int64 * len(device_ids))(*device_ids)
            rc = lib.axon_start_nrt_profile(ids, len(device_ids))
        else:
            rc = lib.axon_start_nrt_profile(None, 0)
        if rc != 0:
            raise RuntimeError(f'axon_start_nrt_profile rc={rc}')
        try:
            yield
        finally:
            n = lib.axon_stop_nrt_profile(str(output_dir).encode())
            print(f'ntff profile: {n} file(s) -> {output_dir}', flush=True)

    mod = types.ModuleType('antenv.axon_hooks')
    mod.get_axon_ntff_profile_hook = lambda: _hook
    mod.set_axon_ntff_profile_hook = lambda h: None
    sys.modules['antenv.axon_hooks'] = mod
    import concourse.bass_utils as bu
    bu.upload_artifacts = lambda tmpdir: 'local://' + tmpdir
    return True


def kernel(**inputs) -> np.ndarray:
    x = np.asarray(inputs['x'])
    B, Tt, Hf, Wf, _ = x.shape
    assert (Tt, Hf, Wf) == (T, 200, 200)
    if 'nc' not in _CACHED:
        _CACHED['nc'] = build_kernel(static_unroll=True)
    nc = _CACHED['nc']
    in_maps = [prep_core_inputs(inputs, b, half)
               for b in range(B) for half in range(2)]
    trace = bool(os.environ.get('KERNEL_TRACE')) and _install_ntff_hook()
    res = run_bass_kernel_spmd(nc, in_maps, core_ids=list(range(N_CORES)),
                               trace=trace,
                               tmpdir=os.environ.get('KERNEL_TRACE_DIR') or None)
    global LAST_EXEC_NS
    LAST_EXEC_NS = res.exec_time_ns
    outs = np.zeros((B, T, 2 * S * HBO, S * Wb, 5), np.float32)
    for b in range(B):
        for half in range(2):
            o = res.results[2 * b + half]['out']   # [T, 32, HBO*Wb]
            for t in range(T):
                img = un_s2d_np(o[t].reshape(32, HBO, Wb), 5, FPAD[3],
                                S * HBO, S * Wb)
                if half == 0:
                    outs[b, t, 0:100] = img
                else:
                    outs[b, t, 100:200] = img[::-1]
    if os.environ.get('KERNEL_TIME'):
        LAST_EXEC_NS = _timed_run(nc, in_maps,
                                  iters=int(os.environ.get('KERNEL_TIME_ITERS', '5')))
    return outs


def _timed_run(nc, in_maps, iters=5):
    """Wall-clock the NEFF execution via a non-donating jitted shard_map,
    device-resident inputs, min over iters. Returns ns."""
    import time
    import jax
    from jax.sharding import Mesh, PartitionSpec, NamedSharding
    from jax.experimental.shard_map import shard_map
    from concourse import bass2jax as b2j

    b2j.install_neuronx_cc_hook()
    partition_name = (nc.partition_id_tensor.name
                      if nc.partition_id_tensor else None)
    in_names, out_names, out_avals, zero_outs = [], [], [], []
    for alloc in nc.m.functions[0].allocations:
        if not isinstance(alloc, mybir.MemoryLocationSet):
            continue
        name = alloc.memorylocations[0].name
        if alloc.kind == "ExternalInput":
            if name != partition_name:
                in_names.append(name)
        elif alloc.kind == "ExternalOutput":
            shape = tuple(alloc.tensor_shape)
            npdt = mybir.dt.np(alloc.dtype)
            out_names.append(name)
            out_avals.append(jax.core.ShapedArray(shape, npdt))
            zero_outs.append(np.zeros(shape, npdt))
    n_params = len(in_names)
    in_names = in_names + out_names
    if partition_name is not None:
        in_names.append(partition_name)

    def _body(*args):
        operands = list(args)
        if partition_name is not None:
            operands.append(b2j.partition_id_tensor())
        outs = b2j._bass_exec_p.bind(
            *operands, out_avals=tuple(out_avals), in_names=tuple(in_names),
            out_names=tuple(out_names), lowering_input_output_aliases=(),
            sim_require_finite=True, sim_require_nnan=True, nc=nc)
        return tuple(outs)

    n = len(in_maps)
    devices = jax.devices()[:n]
    mesh = Mesh(np.asarray(devices), ("core",))
    sh = NamedSharding(mesh, PartitionSpec("core"))
    args = [jax.device_put(
                np.concatenate([np.asarray(in_maps[c][nm]) for c in range(n)], axis=0), sh)
            for nm in in_names[:n_params]]
    args += [jax.device_put(np.concatenate([z] * n, axis=0), sh) for z in zero_outs]
    f = jax.jit(shard_map(_body, mesh=mesh,
                          in_specs=(PartitionSpec("core"),) * (n_params + len(out_names)),
                          out_specs=(PartitionSpec("core"),) * len(out_names),
                          check_rep=False),
                keep_unused=True)
    ts = []
    for _ in range(iters + 1):
        t0 = time.perf_counter()
        o = f(*args)
        jax.block_until_ready(o)
        ts.append(time.perf_counter() - t0)
    best = min(ts[1:])
    print(f'timed_run wall times (s): {[f"{x:.4f}" for x in ts]}', flush=True)
    return int(best * 1e9)


# revision 21
# speedup vs baseline: 12.6078x; 1.0698x over previous
"""Trainium2 Bass kernel for stacked ConvLSTM2D (4 layers, Keras semantics).

Scheme: space-to-depth s=2 block layout; each conv is a sum of block-tap
matmuls with K padded to 128 via shift-baked replica buffers (all matmuls
K=128, M=128, N=CR*Wb, bf16 in / f32 PSUM).

8-core SPMD: batch b on core pair (2b, 2b+1), split over image rows.
Odd cores solve a vertically-FLIPPED half (flipped x + flipped weight slabs
prepped on host) so the program is rank-symmetric: every core owns local
block rows 0..49 (L1: 0..59), its local top is a true image boundary, and
its exchange edge is its local bottom.
 - L1 runs 60 rows with no exchange: validity of the extra 10 overlap rows
   decays 1 row/step; after 10 steps rows <=50 are still valid, exactly
   what L2 needs.
 - L2..L4 exchange a 4-block-row halo every step: each core sends its rows
   46..49 (siy-swapped via two partition-block DMAs; channel order is
   siy-major so the swap is two contiguous partition ranges), pair
   AllReduce(add) in DRAM, then halo = sum - own (exact peer recovery),
   written row-reversed into frame rows 54..57.
 - hrep rebuild is split into main/halo DMAs so only the boundary chunk
   (ci=9) depends on the exchange; chunks 0..8 of the next timestep keep
   the PE busy while the collective flies.
Gates: M-order (i, f, g, o), gate blocks of 4*Fpad partitions, in-block
order (soy, f, sox). State c stays f32 in SBUF; h is bf16.
"""
import math
import os
from contextlib import ExitStack

import numpy as np
import ml_dtypes

import concourse.bacc as bacc
import concourse.bass as bass
import concourse.mybir as mybir
from concourse.tile import TileContext
from concourse.bass_utils import run_bass_kernel_spmd

BF16 = mybir.dt.bfloat16
F32 = mybir.dt.float32
AF = mybir.ActivationFunctionType
ALU = mybir.AluOpType

S = 2
PB = 4
# (cin_raw, F, k, tap_radius R)
LAYERS = [(1, 8, 3, 1), (8, 16, 5, 1), (16, 16, 9, 2), (16, 5, 12, 3)]
FPAD = [8, 16, 16, 8]
CINPAD = [1, 8, 16, 16]
N_CORES = 8
RG = [[0, 1], [2, 3], [4, 5], [6, 7]]

Wb = 100
WbP = Wb + 2 * PB          # 108
HBO1 = 60                  # L1 owned block rows per core
HP1 = HBO1 + 2 * PB        # 68
FLAT1 = HP1 * WbP          # 7344
HBO = 50                   # L2..L4 owned block rows per core
HP2 = HBO + 2 * PB         # 58
FLAT2 = HP2 * WbP          # 6264
HS = (PB + HBO) * WbP      # 5832, halo start (frame row 54)
CR = 5
T = 10


# ---------------------------------------------------------------- host prep --

def same_pad_lo(k):
    return (k - 1) // 2


def s2d_np(img):
    """[H, W, C] -> [4C, Hb, Wb], channel = siy*2C + c*2 + six (siy-major)."""
    H, W, C = img.shape
    Hb, Wbl = H // S, W // S
    t = img.reshape(Hb, S, Wbl, S, C)          # hb, siy, wb, six, c
    return t.transpose(1, 4, 3, 0, 2).reshape(4 * C, Hb, Wbl)


def un_s2d_np(blk, C, Cp, H, W):
    """blk [4Cp(siy,f,six), Hb, Wb] -> [H, W, C] (first C of Cp channels)."""
    Hb, Wbl = H // S, W // S
    b = blk.reshape(S, Cp, S, Hb, Wbl)[:, :C]  # siy, c, six, hb, wb
    return b.transpose(3, 0, 4, 2, 1).reshape(H, W, C)


def remap_kernel(Wk, cin_pad, F, Fp):
    k = Wk.shape[0]
    cin = Wk.shape[2]
    out = np.zeros((k, k, cin_pad, 4 * Fp), np.float32)
    for g in range(4):
        out[:, :, :cin, g * Fp:g * Fp + F] = Wk[:, :, :, g * F:(g + 1) * F]
    return out


def block_weights(Wk, pt, pl, R):
    """Wk [k,k,cinp,4Fp] -> dict[(by,bx)] of [4cinp, 16Fp] with
    row = siy*2cinp + ci*2 + six, col = g*4Fp + soy*2Fp + f*2 + sox."""
    k, _, cinp, coutp = Wk.shape
    Fp = coutp // 4
    out = {}
    for by in range(-R, R + 1):
        for bx in range(-R, R + 1):
            M = np.zeros((4 * cinp, 4 * coutp), np.float32)
            Mv = M.reshape(4 * cinp, 4, S, Fp, S)
            for siy in range(S):
                for six in range(S):
                    for soy in range(S):
                        for sox in range(S):
                            dy = S * by + siy - soy + pt
                            dx = S * bx + six - sox + pl
                            if 0 <= dy < k and 0 <= dx < k:
                                rows = slice(siy * 2 * cinp + six,
                                             siy * 2 * cinp + 2 * cinp, 2)
                                Mv[rows, :, soy, :, sox] = \
                                    Wk[dy, dx].reshape(cinp, 4, Fp)
            out[(by, bx)] = M
    return out


def conv_groups(li, conv):
    """Returns (K, reps, groups) where groups = [(bdy, bdx0)]."""
    R = LAYERS[li][3]
    K = 4 * (CINPAD[li] if conv == 'x' else FPAD[li])
    reps = 128 // K
    ngroups_x = math.ceil((2 * R + 1) / reps)
    groups = [(bdy, -R + m * reps)
              for bdy in range(-R, R + 1) for m in range(ngroups_x)]
    return K, reps, groups


def build_slabs(Wblk, li, conv):
    """-> np [nz*G, 128, 128] f32 (caller casts to bf16). Order: [zh][group]."""
    R = LAYERS[li][3]
    K, reps, groups = conv_groups(li, conv)
    nz = 2 if FPAD[li] == 16 else 1
    slabs = []
    for zh in range(nz):
        for (bdy, bdx0) in groups:
            slab = np.zeros((128, 128), np.float32)
            for j in range(reps):
                bdx = bdx0 + j
                if bdx > R:
                    continue
                slab[j * K:(j + 1) * K, :] = Wblk[(bdy, bdx)][:, zh * 128:(zh + 1) * 128]
            slabs.append(slab)
    return np.stack(slabs)


def prep_core_inputs(inputs, b, half):
    """Build the input map for core 2b+half (half 1 = vertically flipped)."""
    m = {}
    for li, (cin, F, k, R) in enumerate(LAYERS):
        pl = same_pad_lo(k)
        pt = pl if half == 0 else k - 1 - pl
        Fp = FPAD[li]
        Wx = np.asarray(inputs[f'Wx{li+1}'], np.float32)
        Wh = np.asarray(inputs[f'Wh{li+1}'], np.float32)
        if half == 1:
            Wx, Wh = Wx[::-1].copy(), Wh[::-1].copy()
        Wxb = block_weights(remap_kernel(Wx, CINPAD[li], F, Fp), pt, pl, R)
        Whb = block_weights(remap_kernel(Wh, Fp, F, Fp), pt, pl, R)
        braw = np.asarray(inputs[f'b{li+1}'], np.float32)
        bex = np.zeros(16 * Fp, np.float32)
        bexv = bex.reshape(4, S, Fp, S)
        for g in range(4):
            for f in range(F):
                bexv[g, :, f, :] = braw[g * F + f]
        NFp = 4 * Fp
        if Fp == 16:
            b1 = 0.2 * bex[:2 * NFp] + 0.5                             # (i,f) hsig'
            b2 = np.concatenate([bex[2 * NFp:3 * NFp],                 # g raw
                                 0.2 * bex[3 * NFp:] + 0.5])           # o hsig'
            m[f'bias{li+1}a'] = b1.reshape(128, 1)
            m[f'bias{li+1}b'] = b2.reshape(128, 1)
        else:
            b1 = np.concatenate([0.2 * bex[:2 * NFp] + 0.5,            # i,f
                                 bex[2 * NFp:3 * NFp],                 # g raw
                                 0.2 * bex[3 * NFp:] + 0.5])           # o
            m[f'bias{li+1}a'] = b1.reshape(128, 1)
        if li == 0:
            # L1 x: single K=36 im2col slab (tap-major rows), 1 group
            slab = np.zeros((128, 128), np.float32)
            for t_i, (bdy, bdx) in enumerate(
                    (by, bx) for by in range(-R, R + 1) for bx in range(-R, R + 1)):
                slab[t_i * 4:(t_i + 1) * 4, :] = Wxb[(bdy, bdx)]
            m['wx1'] = slab[None].astype(ml_dtypes.bfloat16)
        else:
            m[f'wx{li+1}'] = build_slabs(Wxb, li, 'x').astype(ml_dtypes.bfloat16)
        m[f'wh{li+1}'] = build_slabs(Whb, li, 'h').astype(ml_dtypes.bfloat16)

    # L1 x im2col: local 120 pixel rows (60 block rows), frame 68 block rows
    x = np.asarray(inputs['x'], np.float32)[b]          # [T, 200, 200, 1]
    if half == 1:
        x = x[:, ::-1]
    xc = np.zeros((T, 36, FLAT1 + 8), np.float32)
    for t in range(T):
        xp = np.zeros((4, HP1, WbP), np.float32)
        xp[:, PB:PB + HBO1, PB:PB + Wb] = s2d_np(x[t, 0:2 * HBO1])
        flat = xp.reshape(4, FLAT1)
        for t_i, (bdy, bdx) in enumerate(
                (by, bx) for by in (-1, 0, 1) for bx in (-1, 0, 1)):
            sh = bdy * WbP + bdx
            for c in range(4):
                if sh >= 0:
                    xc[t, t_i * 4 + c, :FLAT1 - sh] = flat[c, sh:]
                else:
                    xc[t, t_i * 4 + c, -sh:FLAT1] = flat[c, :FLAT1 + sh]
    m['xcol'] = xc.astype(ml_dtypes.bfloat16)
    return m


# ------------------------------------------------------------- kernel build --

def build_kernel(static_unroll=True):
    nc = bacc.Bacc("TRN2", target_bir_lowering=False, debug=False,
                   num_devices=N_CORES)

    xcol = nc.dram_tensor('xcol', [T, 36, FLAT1 + 8], BF16, kind="ExternalInput")
    wts, biases = {}, {}
    for li in range(4):
        nz = 2 if FPAD[li] == 16 else 1
        Kx, repx, gx = conv_groups(li, 'x')
        Kh, reph, gh = conv_groups(li, 'h')
        Gx = 1 if li == 0 else nz * len(gx)
        Gh = nz * len(gh)
        wts[(li, 'x')] = nc.dram_tensor(f'wx{li+1}', [Gx, 128, 128], BF16, kind="ExternalInput")
        wts[(li, 'h')] = nc.dram_tensor(f'wh{li+1}', [Gh, 128, 128], BF16, kind="ExternalInput")
        biases[(li, 'a')] = nc.dram_tensor(f'bias{li+1}a', [128, 1], F32, kind="ExternalInput")
        if nz == 2:
            biases[(li, 'b')] = nc.dram_tensor(f'bias{li+1}b', [128, 1], F32, kind="ExternalInput")
    hseqs = [nc.dram_tensor(f'hseq{li+1}', [T + 1, 4 * FPAD[li], FLAT2 + 8], BF16,
                            kind="Internal")
             for li in range(3)]
    out = nc.dram_tensor('out', [T, 32, HBO * Wb], F32, kind="ExternalOutput")

    with TileContext(nc) as tc, ExitStack() as top:
        gp = top.enter_context(tc.tile_pool(name="glob", bufs=1))
        xrA = gp.tile([128, FLAT1], BF16, tag="xrA")
        xrB = gp.tile([128, FLAT1], BF16, tag="xrB")
        hrA = gp.tile([128, FLAT1], BF16, tag="hrA")
        hrB = gp.tile([128, FLAT1], BF16, tag="hrB")
        nc.vector.memset(xrA[:, :], 0.0)
        nc.vector.memset(xrB[:, :], 0.0)

        # all layers' weights + biases loaded up-front (kills layer-boundary
        # PE stalls waiting on slab DMAs)
        wxts, whts, bias_t = {}, {}, {}
        for li in range(4):
            Gx = wts[(li, 'x')].shape[0]
            Gh = wts[(li, 'h')].shape[0]
            wxts[li] = gp.tile([128, Gx * 128], BF16, tag=f"wx{li}", name=f"wxt{li}")
            whts[li] = gp.tile([128, Gh * 128], BF16, tag=f"wh{li}", name=f"wht{li}")
            nc.sync.dma_start(wxts[li][:, :].rearrange("p (g c) -> p g c", c=128),
                              wts[(li, 'x')].ap().rearrange("g p c -> p g c"))
            nc.sync.dma_start(whts[li][:, :].rearrange("p (g c) -> p g c", c=128),
                              wts[(li, 'h')].ap().rearrange("g p c -> p g c"))
            bias_t[(li, 'a')] = gp.tile([128, 1], F32, tag=f"ba{li}", name=f"bat{li}")
            nc.sync.dma_start(bias_t[(li, 'a')][:, :], biases[(li, 'a')].ap())
            if FPAD[li] == 16:
                bias_t[(li, 'b')] = gp.tile([128, 1], F32, tag=f"bb{li}", name=f"bbt{li}")
                nc.sync.dma_start(bias_t[(li, 'b')][:, :], biases[(li, 'b')].ap())

        for li in range(4):
            cin, F, k, R = LAYERS[li]
            Fp = FPAD[li]
            NFp = 4 * Fp
            nz = 2 if Fp == 16 else 1
            Kx, repx, gx = conv_groups(li, 'x')
            Kh, reph, gh = conv_groups(li, 'h')
            if li == 0:
                gx_list = [[(0, 0, 0)]]  # shifts baked into xcol data
            else:
                gx_list = [[(zh * len(gx) + i, bdy, bdx0)
                            for i, (bdy, bdx0) in enumerate(gx)] for zh in range(nz)]
            gh_list = [[(zh * len(gh) + i, bdy, bdx0)
                        for i, (bdy, bdx0) in enumerate(gh)] for zh in range(nz)]

            FLATl = FLAT1 if li == 0 else FLAT2
            HBOl = HBO1 if li == 0 else HBO
            NCHl = HBOl // CR

            NG = 6 if li == 0 else 5          # chunks per vector group
            NGR = NG * CR                     # rows per group

            with ExitStack() as ls:
                lp = ls.enter_context(tc.tile_pool(name=f"l{li}", bufs=1))
                pp = ls.enter_context(tc.tile_pool(name=f"ps{li}", bufs=4, space="PSUM"))
                tp = ls.enter_context(tc.tile_pool(name=f"tmp{li}", bufs=2))
                if li > 0:
                    dp = ls.enter_context(tc.tile_pool(name=f"xch{li}", bufs=2,
                                                       space="DRAM"))

                wxt, wht = wxts[li], whts[li]
                bia = bias_t[(li, 'a')]
                if nz == 2:
                    bib = bias_t[(li, 'b')]
                # gate staging (scalar engine drains PSUM into these)
                PA = 2 * NFp                          # i,f partitions
                PG = NFp                              # g (and o) partitions
                As = lp.tile([128, HBOl, Wb], BF16, tag="As")
                Gs = lp.tile([128, HBOl, Wb], BF16, tag="Gs")
                Os = lp.tile([128, HBOl, Wb], BF16, tag="Os")
                TCs = lp.tile([128, HBOl, Wb], BF16, tag="TCs")

                H = lp.tile([NFp, FLATl + 8], BF16, tag="H")
                C = lp.tile([128, HBOl, Wb], F32, tag="C")
                nc.vector.memset(H[:, :], 0.0)
                nc.vector.memset(C[:, :, :], 0.0)
                if li == 3:
                    OS = lp.tile([32, HBO, Wb], F32, tag="OS")
                if li < 3:
                    nc.sync.dma_start(hseqs[li].ap()[0, :, :], H[:, 0:FLAT2 + 8])
                if li > 0:
                    SB = lp.tile([128, 4, WbP], BF16, tag="SB")
                    Rt = lp.tile([128, 4, WbP], BF16, tag="Rt")

                H3 = H[:, 0:FLATl].rearrange("p (h w) -> p h w", w=WbP)

                def build_xr(t, buf):
                    if li == 0:
                        nc.sync.dma_start(buf[0:36, 0:FLAT1],
                                          xcol.ap()[bass.ds(t, 1), :, 0:FLAT1])
                    else:
                        src = hseqs[li - 1].ap()
                        for j in range(repx):
                            nc.sync.dma_start(
                                buf[j * Kx:(j + 1) * Kx, 0:FLAT2],
                                src[bass.ds(t + 1, 1), 0:Kx, j:j + FLAT2])

                def band_range(b):
                    lo = 0 if b == 0 else (PB + b * CR) * WbP
                    if b == NCHl - 1:
                        hi = FLAT1 if li == 0 else HS - 8
                    else:
                        hi = (PB + (b + 1) * CR) * WbP
                    return lo, hi

                def issue_band(b, buf):
                    lo, hi = band_range(b)
                    for j in range(reph):
                        nc.sync.dma_start(buf[j * Kh:(j + 1) * Kh, lo:hi],
                                          H[0:Kh, lo + j:hi + j])

                # initial state (t=0): hrA holds zeros; xrA holds x/hseq slice 0
                nc.vector.memset(hrA[:, :], 0.0)
                build_xr(0, xrA)

                def step_body(t):
                    xr_cur, xr_nxt = (xrA, xrB) if t % 2 == 0 else (xrB, xrA)
                    hr_cur, hr_nxt = (hrA, hrB) if t % 2 == 0 else (hrB, hrA)
                    if t < T - 1:
                        build_xr(t + 1, xr_nxt)
                    xr3 = xr_cur[:, 0:FLATl].rearrange("p (h w) -> p h w", w=WbP)
                    hr3 = hr_cur[:, 0:FLATl].rearrange("p (h w) -> p h w", w=WbP)

                    for ci in range(NCHl):
                        r0 = PB + ci * CR
                        cs = slice(ci * CR, (ci + 1) * CR)
                        zts = []
                        for zh in range(nz):
                            zt = pp.tile([128, CR, Wb], F32, tag="z")
                            mms = [(wxt, xr3, s, bdy, bdx0)
                                   for (s, bdy, bdx0) in gx_list[zh]] + \
                                  [(wht, hr3, s, bdy, bdx0)
                                   for (s, bdy, bdx0) in gh_list[zh]]
                            for mi, (wt, rep3, s, bdy, bdx0) in enumerate(mms):
                                nc.tensor.matmul(
                                    zt[:, :, :],
                                    wt[:, s * 128:(s + 1) * 128],
                                    rep3[:, r0 + bdy:r0 + bdy + CR,
                                         PB + bdx0:PB + bdx0 + Wb],
                                    start=(mi == 0), stop=(mi == len(mms) - 1))
                            zts.append(zt)

                        # scalar engine drains PSUM -> bf16 staging
                        if nz == 2:
                            z1, z2 = zts
                            nc.scalar.activation(As[0:PA, cs, :], z1[:, :, :], AF.Relu,
                                                 bias=bia[:, 0:1], scale=0.2)
                            nc.scalar.activation(Gs[0:PG, cs, :], z2[0:PG, :, :], AF.Tanh,
                                                 bias=bib[0:PG, 0:1], scale=1.0)
                            nc.scalar.activation(Os[0:PG, cs, :], z2[PG:128, :, :], AF.Relu,
                                                 bias=bib[PG:128, 0:1], scale=0.2)
                        else:
                            z = zts[0]
                            nc.scalar.activation(As[0:PA, cs, :], z[0:PA, :, :], AF.Relu,
                                                 bias=bia[0:PA, 0:1], scale=0.2)
                            nc.scalar.activation(Gs[0:PG, cs, :], z[PA:PA + PG, :, :],
                                                 AF.Tanh, bias=bia[PA:PA + PG, 0:1],
                                                 scale=1.0)
                            nc.scalar.activation(Os[0:PG, cs, :], z[PA + PG:128, :, :],
                                                 AF.Relu, bias=bia[PA + PG:128, 0:1],
                                                 scale=0.2)

                        # group tail: big vector ops over NG chunks
                        if (ci + 1) % NG == 0:
                            g = ci // NG
                            gs = slice(g * NGR, (g + 1) * NGR)
                            cg = C[NFp:2 * NFp, gs, :]
                            nc.vector.tensor_scalar_min(As[0:PA, gs, :], As[0:PA, gs, :],
                                                        1.0)
                            nc.vector.tensor_scalar_min(Os[0:PG, gs, :], Os[0:PG, gs, :],
                                                        1.0)
                            tg = tp.tile([128, NGR, Wb], F32, tag="t")
                            tgv = tg[NFp:2 * NFp, :, :]
                            nc.vector.tensor_tensor(tgv, As[0:NFp, gs, :],
                                                    Gs[0:NFp, gs, :], ALU.mult)
                            nc.vector.tensor_tensor(cg, As[NFp:2 * NFp, gs, :], cg,
                                                    ALU.mult)
                            nc.vector.tensor_tensor(cg, cg, tgv, ALU.add)
                            nc.scalar.activation(TCs[0:PG, gs, :], cg, AF.Tanh)
                            hw = H3[0:NFp, PB + g * NGR:PB + (g + 1) * NGR, PB:PB + Wb]
                            nc.vector.tensor_tensor(hw, Os[0:PG, gs, :],
                                                    TCs[0:PG, gs, :], ALU.mult)
                            if li == 3:
                                nc.vector.tensor_tensor(OS[:, gs, :], Os[0:PG, gs, :],
                                                        TCs[0:PG, gs, :], ALU.mult)
                            if t < T - 1:
                                for b in range(max(0, g * NG - 1),
                                               min((g + 1) * NG - 1, NCHl - 1)):
                                    issue_band(b, hr_nxt)
                    if t < T - 1:
                        issue_band(NCHl - 1, hr_nxt)

                    if li == 3:
                        nc.sync.dma_start(
                            out.ap()[bass.ds(t, 1), :, :],
                            OS[:, :, :].rearrange("p h w -> p (h w)"))
                    elif li == 0:
                        nc.sync.dma_start(hseqs[li].ap()[bass.ds(t + 1, 1), :, :],
                                          H[:, 0:FLAT2 + 8])
                    else:
                        # main part of hseq (no halo dependency)
                        nc.sync.dma_start(
                            hseqs[li].ap()[bass.ds(t + 1, 1), :, 0:HS - 8],
                            H[:, 0:HS - 8])

                    # --- halo exchange (L2..L4, skip last step of L4) ---
                    if li > 0 and not (li == 3 and t == T - 1):
                        TFp = 2 * Fp
                        nc.gpsimd.dma_start(SB[0:TFp, :, :],
                                            H3[TFp:NFp, PB + HBO - 4:PB + HBO, :])
                        nc.gpsimd.dma_start(SB[TFp:NFp, :, :],
                                            H3[0:TFp, PB + HBO - 4:PB + HBO, :])
                        sendt = dp.tile([NFp, 4 * WbP], BF16, tag="send")
                        recvt = dp.tile([NFp, 4 * WbP], BF16, tag="recv")
                        nc.gpsimd.dma_start(sendt[:, :],
                                            SB[0:NFp, :, :].rearrange("p h w -> p (h w)"))
                        nc.gpsimd.collective_compute(
                            "AllReduce", ALU.add, replica_groups=RG,
                            ins=[sendt[:, :]], outs=[recvt[:, :]])
                        nc.gpsimd.dma_start(Rt[0:NFp, :, :].rearrange("p h w -> p (h w)"),
                                            recvt[:, :])
                        for r in range(4):
                            nc.vector.tensor_tensor(
                                H3[:, PB + HBO + r, :], Rt[0:NFp, 3 - r, :],
                                SB[0:NFp, 3 - r, :], ALU.subtract)
                        # halo band of hrep for t+1 + halo part of hseq store
                        if t < T - 1:
                            for j in range(reph):
                                nc.gpsimd.dma_start(
                                    hr_nxt[j * Kh:(j + 1) * Kh, HS - 8:FLAT2],
                                    H[0:Kh, HS - 8 + j:FLAT2 + j])
                        if li < 3:
                            nc.gpsimd.dma_start(
                                hseqs[li].ap()[bass.ds(t + 1, 1), :, HS - 8:FLAT2 + 8],
                                H[:, HS - 8:FLAT2 + 8])

                if static_unroll:
                    for t in range(T):
                        step_body(t)
                else:
                    with tc.For_i(0, T) as t:
                        step_body(t)
    nc.compile()
    return nc


# ------------------------------------------------------------------ runner --

_CACHED = {}
LAST_EXEC_NS = None


def _install_ntff_hook():
    """Provide the antenv.axon_hooks module this image lacks, backed by
    ctypes calls into libaxon_pjrt.so (same ABI trn_boot would use)."""
    import sys
    import types
    import ctypes
    import contextlib
    if 'antenv.axon_hooks' in sys.modules:
        return True
    try:
        lib = ctypes.CDLL('/opt/axon/libaxon_pjrt.so')
    except OSError:
        return False
    if not hasattr(lib, 'axon_start_nrt_profile'):
        return False
    lib.axon_start_nrt_profile.argtypes = [ctypes.POINTER(ctypes.c_int64),
                                           ctypes.c_size_t]
    lib.axon_start_nrt_profile.restype = ctypes.c_int64
    lib.axon_stop_nrt_profile.argtypes = [ctypes.c_char_p]
    lib.axon_stop_nrt_profile.restype = ctypes.c_int64

    @contextlib.contextmanager
    def _hook(output_dir, device_ids):
        import jax
        jax.devices()
        if device_ids:
            ids = (ctypes.c_int64 * len(device_ids))(*device_ids)
            rc = lib.axon_start_nrt_profile(ids, len(device_ids))
        else:
            rc = lib.axon_start_nrt_profile(None, 0)
        if rc != 0:
            raise RuntimeError(f'axon_start_nrt_profile rc={rc}')
        try:
            yield
        finally:
            n = lib.axon_stop_nrt_profile(str(output_dir).encode())
            print(f'ntff profile: {n} file(s) -> {output_dir}', flush=True)

    mod = types.ModuleType('antenv.axon_hooks')
    mod.get_axon_ntff_profile_hook = lambda: _hook
    mod.set_axon_ntff_profile_hook = lambda h: None
    sys.modules['antenv.axon_hooks'] = mod
    import concourse.bass_utils as bu
    bu.upload_artifacts = lambda tmpdir: 'local://' + tmpdir
    return True


def kernel(**inputs) -> np.ndarray:
    x = np.asarray(inputs['x'])
    B, Tt, Hf, Wf, _ = x.shape
    assert (Tt, Hf, Wf) == (T, 200, 200)
    if 'nc' not in _CACHED:
        _CACHED['nc'] = build_kernel(static_unroll=True)
    nc = _CACHED['nc']
    in_maps = [prep_core_inputs(inputs, b, half)
               for b in range(B) for half in range(2)]
    trace = bool(os.environ.get('KERNEL_TRACE')) and _install_ntff_hook()
    res = run_bass_kernel_spmd(nc, in_maps, core_ids=list(range(N_CORES)),
                               trace=trace,
                               tmpdir=os.environ.get('KERNEL_TRACE_DIR') or None)
    global LAST_EXEC_NS
    LAST_EXEC_NS = res.exec_time_ns
    outs = np.zeros((B, T, 2 * S * HBO, S * Wb, 5), np.float32)
    for b in range(B):
        for half in range(2):
            o = res.results[2 * b + half]['out']   # [T, 32, HBO*Wb]
            for t in range(T):
                img = un_s2d_np(o[t].reshape(32, HBO, Wb), 5, FPAD[3],
                                S * HBO, S * Wb)
                if half == 0:
                    outs[b, t, 0:100] = img
                else:
                    outs[b, t, 100:200] = img[::-1]
    if os.environ.get('KERNEL_TIME'):
        LAST_EXEC_NS = _timed_run(nc, in_maps,
                                  iters=int(os.environ.get('KERNEL_TIME_ITERS', '5')))
    return outs


def _timed_run(nc, in_maps, iters=5):
    """Wall-clock the NEFF execution via a non-donating jitted shard_map,
    device-resident inputs, min over iters. Returns ns."""
    import time
    import jax
    from jax.sharding import Mesh, PartitionSpec, NamedSharding
    from jax.experimental.shard_map import shard_map
    from concourse import bass2jax as b2j

    b2j.install_neuronx_cc_hook()
    partition_name = (nc.partition_id_tensor.name
                      if nc.partition_id_tensor else None)
    in_names, out_names, out_avals, zero_outs = [], [], [], []
    for alloc in nc.m.functions[0].allocations:
        if not isinstance(alloc, mybir.MemoryLocationSet):
            continue
        name = alloc.memorylocations[0].name
        if alloc.kind == "ExternalInput":
            if name != partition_name:
                in_names.append(name)
        elif alloc.kind == "ExternalOutput":
            shape = tuple(alloc.tensor_shape)
            npdt = mybir.dt.np(alloc.dtype)
            out_names.append(name)
            out_avals.append(jax.core.ShapedArray(shape, npdt))
            zero_outs.append(np.zeros(shape, npdt))
    n_params = len(in_names)
    in_names = in_names + out_names
    if partition_name is not None:
        in_names.append(partition_name)

    def _body(*args):
        operands = list(args)
        if partition_name is not None:
            operands.append(b2j.partition_id_tensor())
        outs = b2j._bass_exec_p.bind(
            *operands, out_avals=tuple(out_avals), in_names=tuple(in_names),
            out_names=tuple(out_names), lowering_input_output_aliases=(),
            sim_require_finite=True, sim_require_nnan=True, nc=nc)
        return tuple(outs)

    n = len(in_maps)
    devices = jax.devices()[:n]
    mesh = Mesh(np.asarray(devices), ("core",))
    sh = NamedSharding(mesh, PartitionSpec("core"))
    args = [jax.device_put(
                np.concatenate([np.asarray(in_maps[c][nm]) for c in range(n)], axis=0), sh)
            for nm in in_names[:n_params]]
    args += [jax.device_put(np.concatenate([z] * n, axis=0), sh) for z in zero_outs]
    f = jax.jit(shard_map(_body, mesh=mesh,
                          in_specs=(PartitionSpec("core"),) * (n_params + len(out_names)),
                          out_specs=(PartitionSpec("core"),) * len(out_names),
                          check_rep=False),
                keep_unused=True)
    ts = []
    for _ in range(iters + 1):
        t0 = time.perf_counter()
        o = f(*args)
        jax.block_until_ready(o)
        ts.append(time.perf_counter() - t0)
    best = min(ts[1:])
    print(f'timed_run wall times (s): {[f"{x:.4f}" for x in ts]}', flush=True)
    return int(best * 1e9)
